# revision 1
# baseline (speedup 1.0000x reference)
"""Bass/Tile kernel builder for DeformConv (one sample per NeuronCore).

Index conventions:
  l = i * 128 + p   (p = SBUF partition, i = l-block 0..31)
  image row = l // 64, col = l % 64;  tap k = ky*3 + kx
  base arrays [128 p, 9 k, 32 i] fp32
  corner weights w4 [128, 4 r, 9 k, 32 i] fp32
  pix16 [128, 18 t, 32 i] int16, t = 2k + (0 top / 1 bottom pair)
  xtd DRAM [4224, 128] fp16, row = pixel + 64 (guard bands zeroed)
  gather t: list position j = i_local*128 + p -> idx[q=j%16, s=j//16=i_local*8+p//16]
"""
from contextlib import ExitStack

import numpy as np

import concourse.bass as bass
import concourse.mybir as mybir
import concourse.tile as tile
from concourse import masks

F32 = mybir.dt.float32
F32R = mybir.dt.float32r
F16 = mybir.dt.float16
I16 = mybir.dt.int16
ALU = mybir.AluOpType
ACTF = mybir.ActivationFunctionType

Cin = Cout = 128
HW = 4096
NTAP = 9
GUARD = 64
NROWS = HW + 2 * GUARD
NBLK = 32
NHALF = 2
BPH = NBLK // NHALF  # blocks per half

# stage toggles for cost attribution (dev only)
CFG = {"gather": True, "combine": True, "transpose": True, "matmul": True}


def host_constants():
    l = np.arange(HW)
    p = l % 128
    blk = l // 128
    i_img = l // 64
    j_img = l % 64
    ky = np.arange(9) // 3
    kx = np.arange(9) % 3
    basepy = np.zeros((128, 9, 32), np.float32)
    basepx = np.zeros((128, 9, 32), np.float32)
    for k in range(9):
        basepy[p, k, blk] = i_img - 1 + ky[k]
        basepx[p, k, blk] = j_img - 1 + kx[k]
    return {"basepy": basepy, "basepx": basepx}


def host_weights(w_offset, w):
    wofft = np.ascontiguousarray(
        w_offset.transpose(2, 3, 1, 0).reshape(9, 128, 18)).astype(np.float16)
    wmainT = np.ascontiguousarray(
        w.transpose(2, 3, 1, 0).reshape(9, 128, 128)).astype(np.float16)
    return {"wofft": wofft, "wmainT": wmainT}


def declare_io(nc, debug=False):
    io = {}
    io["xin"] = nc.dram_tensor("xin", (128, HW), F32, kind="ExternalInput").ap()
    io["wofft"] = nc.dram_tensor("wofft", (9, 128, 18), F16, kind="ExternalInput").ap()
    io["wmainT"] = nc.dram_tensor("wmainT", (9, 128, 128), F16, kind="ExternalInput").ap()
    io["basepy"] = nc.dram_tensor("basepy", (128, 9, 32), F32, kind="ExternalInput").ap()
    io["basepx"] = nc.dram_tensor("basepx", (128, 9, 32), F32, kind="ExternalInput").ap()
    io["out"] = nc.dram_tensor("out", (128, HW), F32, kind="ExternalOutput").ap()
    io["xtd"] = nc.dram_tensor("xtd", (NROWS, 128), F16,
                               kind="ExternalOutput" if debug else "Internal").ap()
    io["pixb"] = nc.dram_tensor("pixb", (128, 9 * 32), I16,
                                kind="ExternalOutput" if debug else "Internal").ap()
    io["xpair"] = nc.dram_tensor("xpair", (4352, 2, 128), F16, kind="Internal").ap()
    io["debug"] = debug
    if debug:
        io["d_offsb"] = nc.dram_tensor("d_offsb", (18, HW), F32, kind="ExternalOutput").ap()
        io["d_offT"] = nc.dram_tensor("d_offT", (128, 32 * 18), F32, kind="ExternalOutput").ap()
        io["d_w4"] = nc.dram_tensor("d_w4", (128, 4 * 9 * 32), F32, kind="ExternalOutput").ap()
        io["d_idxw"] = nc.dram_tensor("d_idxw", (128, 9 * 256), I16, kind="ExternalOutput").ap()
        io["d_gtop"] = nc.dram_tensor("d_gtop", (128, BPH * 256), F16, kind="ExternalOutput").ap()
        io["d_gbot"] = nc.dram_tensor("d_gbot", (128, BPH * 256), F16, kind="ExternalOutput").ap()
        io["d_sampT"] = nc.dram_tensor("d_sampT", (128, BPH * 128), F16, kind="ExternalOutput").ap()
        io["d_sampN"] = nc.dram_tensor("d_sampN", (128, BPH * 128), F16, kind="ExternalOutput").ap()
        for nm in ("d_py", "d_fy", "d_y0", "d_x0", "d_fx", "d_my0", "d_a0"):
            io[nm] = nc.dram_tensor(nm, (128, 32), F32, kind="ExternalOutput").ap()
    return io


def build(tc: tile.TileContext, io: dict):
    nc = tc.nc
    xin, wofft, wmainT = io["xin"], io["wofft"], io["wmainT"]
    basepy, basepx, out = io["basepy"], io["basepx"], io["out"]
    xtd, pixb, xpair = io["xtd"], io["pixb"], io["xpair"]

    ctx = ExitStack()
    const = ctx.enter_context(tc.tile_pool(name="const", bufs=1))
    persist = ctx.enter_context(tc.tile_pool(name="persist", bufs=1))
    coord = ctx.enter_context(tc.tile_pool(name="coord", bufs=2))
    evac = ctx.enter_context(tc.tile_pool(name="evac", bufs=3))

    ident32 = const.tile([128, 128], F32)
    masks.make_identity(nc, ident32[:])
    ident16 = const.tile([128, 128], F16)
    masks.make_identity(nc, ident16[:])
    zeros16 = const.tile([128, 64], F16)
    nc.vector.memset(zeros16[:], 0.0)

    xtd_flat = xtd.rearrange("r c -> (r c)")
    ng = GUARD * 128
    nc.sync.dma_start(out=xtd_flat[0:ng].rearrange("(p n) -> p n", p=128),
                      in_=zeros16[:])
    nc.sync.dma_start(out=xtd_flat[(GUARD + HW) * 128:].rearrange("(p n) -> p n", p=128),
                      in_=zeros16[:])

    xpad = persist.tile([128, 66, 66], F16)
    nc.vector.memset(xpad[:], 0.0)
    nc.gpsimd.dma_start(out=xpad[:, 1:65, 1:65],
                        in_=xin.rearrange("c (h w) -> c h w", h=64))
    # column-shifted contiguous copies: xsh[:, kx, r, j] = xpad[c, r, j+kx]
    xsh = persist.tile([128, 3, 66 * 64], F16)
    for kx in range(3):
        nc.vector.tensor_copy(
            xsh[:, kx, :].rearrange("p (r j) -> p r j", r=66),
            xpad[:, :, kx:kx + 64])

    wofft_sb = persist.tile([128, 9, 18], F16)
    nc.sync.dma_start(out=wofft_sb[:], in_=wofft.rearrange("k c f -> c k f"))
    wmainT_sb = persist.tile([128, 9, 128], F16)
    nc.sync.dma_start(out=wmainT_sb[:], in_=wmainT.rearrange("k c o -> c k o"))
    basepy_sb = persist.tile([128, 9, 32], F32)
    nc.sync.dma_start(out=basepy_sb[:], in_=basepy)
    basepx_sb = persist.tile([128, 9, 32], F32)
    nc.sync.dma_start(out=basepx_sb[:], in_=basepx)

    offsb = persist.tile([18, HW], F32)
    offT = persist.tile([128, 32, 18], F32)
    w4 = persist.tile([128, 4, 9, 32], F32)
    pix16 = persist.tile([128, 9, 32], I16)
    idxw = persist.tile([128, 9, 256], I16)

    # ---------------- prologue (own PSUM scope) ----------------
    with tc.tile_pool(name="prepsum", bufs=2, space="PSUM") as pps:
        # offset conv (fp16 in, fp32 psum)
        for nb in range(8):
            ps = pps.tile([18, 512], F32, tag="psoff")
            for k in range(NTAP):
                ky, kx = k // 3, k % 3
                r0 = (nb * 8 + ky) * 64
                rhs = xsh[:, kx, r0:r0 + 512]
                nc.tensor.matmul(ps[:], wofft_sb[:, k, :], rhs,
                                 start=(k == 0), stop=(k == NTAP - 1))
            nc.scalar.activation(offsb[:, nb * 512:(nb + 1) * 512], ps[:], ACTF.Copy)

        # transpose offsets -> offT
        for i in range(NBLK):
            pst = pps.tile([128, 18], F32, tag="pst")
            nc.tensor.transpose(pst[:], offsb[:, i * 128:(i + 1) * 128],
                                ident32[0:18, 0:18])
            nc.scalar.activation(offT[:, i, :], pst[:], ACTF.Copy)

        # xT build (fp16 transpose)
        for i in range(NBLK):
            psx = pps.tile([128, 128], F16, tag="psx")
            r0 = (2 * i + 1) * 64
            nc.tensor.transpose(psx[:], xsh[:, 1, r0:r0 + 128], ident16[:])
            xts = evac.tile([128, 128], F16, tag="xts")
            nc.scalar.activation(xts[:], psx[:], ACTF.Copy)
            dst = xtd_flat[(GUARD + 128 * i) * 128:(GUARD + 128 * (i + 1)) * 128]
            nc.sync.dma_start(out=dst.rearrange("(p n) -> p n", p=128), in_=xts[:])

    # ---------------- xpair build (DRAM->DRAM interleave) ----------------
    # xpair[r, 0, :] = xtd[r - 64]  for r in [64, 4288)
    # xpair[r, 1, :] = xtd[r]       for r in [0, 4224)
    xpair_flat = xpair.rearrange("r s c -> (r s c)")
    nc.sync.dma_start(
        out=bass.AP(xpair_flat.tensor, 64 * 256, [[256, NROWS], [1, 128]]),
        in_=xtd)
    nc.sync.dma_start(
        out=bass.AP(xpair_flat.tensor, 128, [[256, NROWS], [1, 128]]),
        in_=xtd)
    # zero uncovered cells: slot0 rows [0,64)+[4288,4352); slot1 rows [4224,4352)
    for off in (0, 4288 * 256, 4224 * 256 + 128, 4288 * 256 + 128):
        nc.sync.dma_start(out=bass.AP(xpair_flat.tensor, off, [[256, 64], [1, 128]]),
                          in_=zeros16[:])

    # ---------------- coords / weights / indices (DVE) ----------------
    for k in range(NTAP):
        dy = offT[:, :, 2 * k]
        dx = offT[:, :, 2 * k + 1]
        py = coord.tile([128, 32], F32, tag="py")
        nc.vector.tensor_tensor(py[:], dy, basepy_sb[:, k, :], ALU.add)
        px = coord.tile([128, 32], F32, tag="px")
        nc.vector.tensor_tensor(px[:], dx, basepx_sb[:, k, :], ALU.add)
        # exact floor: trunc-cast, then subtract 1 where truncation went up
        def floorfrac(src, tagp):
            ti = coord.tile([128, 32], mybir.dt.int32, tag=tagp + "i", name=tagp + "i")
            nc.vector.tensor_copy(ti[:], src[:])
            tf = coord.tile([128, 32], F32, tag=tagp + "f", name=tagp + "f")
            nc.vector.tensor_copy(tf[:], ti[:])
            gt = coord.tile([128, 32], F32, tag=tagp + "g", name=tagp + "g")
            nc.vector.tensor_tensor(gt[:], tf[:], src[:], ALU.is_gt)
            fl = coord.tile([128, 32], F32, tag=tagp + "fl", name=tagp + "fl")
            nc.vector.tensor_tensor(fl[:], tf[:], gt[:], ALU.subtract)
            fr = coord.tile([128, 32], F32, tag=tagp + "fr", name=tagp + "fr")
            nc.vector.tensor_tensor(fr[:], src[:], fl[:], ALU.subtract)
            return fl, fr

        y0, fy = floorfrac(py, "yy")
        x0, fx = floorfrac(px, "xx")

        def wmask(src, lo, hi, tag):
            m1 = coord.tile([128, 32], F32, tag=tag + "a")
            nc.vector.tensor_scalar(m1[:], src[:], float(lo), None, ALU.is_ge)
            m2 = coord.tile([128, 32], F32, tag=tag + "b")
            nc.vector.tensor_scalar(m2[:], src[:], float(hi), None, ALU.is_le)
            m = coord.tile([128, 32], F32, tag=tag)
            nc.vector.tensor_tensor(m[:], m1[:], m2[:], ALU.mult)
            return m

        if io["debug"] and k == 0:
            for nm, tt in (("d_py", py), ("d_fy", fy), ("d_y0", y0), ("d_x0", x0), ("d_fx", fx)):
                nc.sync.dma_start(out=io[nm], in_=tt[:])
        my0 = wmask(y0, 0, 63, "my0")
        my1 = wmask(y0, -1, 62, "my1")
        mx0 = wmask(x0, 0, 63, "mx0")
        mx1 = wmask(x0, -1, 62, "mx1")

        a0 = coord.tile([128, 32], F32, tag="a0")
        nc.vector.tensor_scalar(a0[:], fy[:], -1.0, 1.0, ALU.mult, ALU.add)
        nc.vector.tensor_tensor(a0[:], a0[:], my0[:], ALU.mult)
        if io["debug"] and k == 0:
            nc.sync.dma_start(out=io["d_my0"], in_=my0[:])
            nc.sync.dma_start(out=io["d_a0"], in_=a0[:])
        a1 = coord.tile([128, 32], F32, tag="a1")
        nc.vector.tensor_tensor(a1[:], fy[:], my1[:], ALU.mult)
        b0 = coord.tile([128, 32], F32, tag="b0")
        nc.vector.tensor_scalar(b0[:], fx[:], -1.0, 1.0, ALU.mult, ALU.add)
        nc.vector.tensor_tensor(b0[:], b0[:], mx0[:], ALU.mult)
        b1 = coord.tile([128, 32], F32, tag="b1")
        nc.vector.tensor_tensor(b1[:], fx[:], mx1[:], ALU.mult)

        nc.vector.tensor_tensor(w4[:, 0, k, :], a0[:], b0[:], ALU.mult)
        nc.vector.tensor_tensor(w4[:, 1, k, :], a0[:], b1[:], ALU.mult)
        nc.vector.tensor_tensor(w4[:, 2, k, :], a1[:], b0[:], ALU.mult)
        nc.vector.tensor_tensor(w4[:, 3, k, :], a1[:], b1[:], ALU.mult)

        pixf = coord.tile([128, 32], F32, tag="pixf")
        nc.vector.tensor_scalar(pixf[:], y0[:], 64.0, None, ALU.mult)
        nc.vector.tensor_tensor(pixf[:], pixf[:], x0[:], ALU.add)
        pt = coord.tile([128, 32], F32, tag="pt")
        nc.vector.tensor_scalar(pt[:], pixf[:], -128.0, 4222.0, ALU.max, ALU.min)
        nc.vector.tensor_scalar(pt[:], pt[:], 128.0, None, ALU.add)
        nc.vector.tensor_copy(pix16[:, k, :], pt[:])

    # ---------------- idx wrap via DRAM bounce ----------------
    nc.sync.dma_start(out=pixb, in_=pix16[:].rearrange("p t i -> p (t i)"))
    pixb_flat = pixb.rearrange("p n -> (p n)")
    dst0 = idxw[0:16, :, :].rearrange("q t (i h) -> q t i h", h=8)
    src0 = bass.AP(pixb_flat.tensor, 0,
                   [[288, 16], [32, 9], [1, 32], [16 * 288, 8]])
    nc.sync.dma_start(out=dst0, in_=src0)
    for g in range(1, 8):
        nc.sync.dma_start(out=idxw[16 * g:16 * (g + 1), :, :], in_=idxw[0:16, :, :])

    if io["debug"]:
        nc.sync.dma_start(out=io["d_offsb"], in_=offsb[:])
        nc.sync.dma_start(out=io["d_offT"], in_=offT[:].rearrange("p a b -> p (a b)"))
        nc.sync.dma_start(out=io["d_w4"], in_=w4[:].rearrange("p a b c -> p (a b c)"))
        nc.sync.dma_start(out=io["d_idxw"], in_=idxw[:].rearrange("p a b -> p (a b)"))

    # ---------------- main loop ----------------
    gather_src = bass.AP(xpair_flat.tensor, 0, [[256, 4351], [1, 512]])

    with tc.tile_pool(name="psout", bufs=1, space="PSUM") as psout, \
         tc.tile_pool(name="pstr", bufs=3, space="PSUM") as pstr, \
         tc.tile_pool(name="gpool", bufs=2) as gpool, \
         tc.tile_pool(name="spool", bufs=2) as spool, \
         tc.tile_pool(name="tpool", bufs=4) as tpool:
        for hf in range(NHALF):
            blk0 = hf * BPH
            pso = [psout.tile([128, 512], F32, tag=f"pso{c}", name=f"pso{c}_{hf}")
                   for c in range(4)]
            for k in range(NTAP):
                gq = gpool.tile([128, BPH, 512], F16, tag="gq")
                if not CFG["gather"]:
                    nc.vector.memset(gq[:], 0.25)
                else:
                    nc.gpsimd.dma_gather(
                        out_ap=gq[:],
                        in_ap=gather_src,
                        idxs_ap=idxw[:, k, blk0 * 8:(blk0 + BPH) * 8],
                        num_idxs=BPH * 128,
                        num_idxs_reg=BPH * 128,
                        elem_size=512,
                        elem_step=256,
                        single_packet=False,
                    )
                sampT = spool.tile([128, BPH, 128], F16, tag="sampT")
                for i in range(BPH):
                    if not CFG["combine"]:
                        nc.vector.tensor_copy(sampT[:, i, :], gtop[:, i, 0:128])
                        continue
                    ib = blk0 + i
                    t0 = tpool.tile([128, 128], F16, tag="t0")
                    nc.vector.tensor_scalar(t0[:], gq[:, i, 0:128],
                                            w4[:, 0, k, ib:ib + 1], None, ALU.mult)
                    t1 = tpool.tile([128, 128], F16, tag="t1")
                    nc.vector.scalar_tensor_tensor(t1[:], gq[:, i, 128:256],
                                                   w4[:, 2, k, ib:ib + 1], t0[:],
                                                   ALU.mult, ALU.add)
                    t2 = tpool.tile([128, 128], F16, tag="t2")
                    nc.vector.scalar_tensor_tensor(t2[:], gq[:, i, 256:384],
                                                   w4[:, 1, k, ib:ib + 1], t1[:],
                                                   ALU.mult, ALU.add)
                    nc.vector.scalar_tensor_tensor(sampT[:, i, :], gq[:, i, 384:512],
                                                   w4[:, 3, k, ib:ib + 1], t2[:],
                                                   ALU.mult, ALU.add)
                sampN = spool.tile([128, BPH * 128], F16, tag="sampN")
                for i4 in range(BPH // 4):
                    if not CFG["transpose"]:
                        for i in range(4 * i4, 4 * i4 + 4):
                            nc.vector.tensor_copy(sampN[:, i * 128:(i + 1) * 128], sampT[:, i, :])
                        continue
                    pss = pstr.tile([128, 512], F16, tag="pss")
                    for j in range(4):
                        i = 4 * i4 + j
                        nc.tensor.transpose(pss[:, j * 128:(j + 1) * 128],
                                            sampT[:, i, :], ident16[:])
                    nc.scalar.activation(sampN[:, i4 * 512:(i4 + 1) * 512], pss[:],
                                         ACTF.Copy)
                if io["debug"] and hf == 0 and k == 0:
                    nc.sync.dma_start(out=io["d_sampT"], in_=sampT[:].rearrange("p a b -> p (a b)"))
                    nc.sync.dma_start(out=io["d_sampN"], in_=sampN[:])
                for c in (range(4) if CFG["matmul"] else ()):
                    nc.tensor.matmul(pso[c][:], wmainT_sb[:, k, :],
                                     sampN[:, c * 512:(c + 1) * 512],
                                     start=(k == 0), stop=(k == NTAP - 1))
            for c in range(4):
                osb = evac.tile([128, 512], F32, tag="osb")
                nc.scalar.activation(osb[:], pso[c][:], ACTF.Copy)
                l0 = hf * 2048 + c * 512
                nc.sync.dma_start(out=out[:, l0:l0 + 512], in_=osb[:])
    ctx.close()


# ======================= runner =======================
import concourse.bacc as _bacc
from concourse.bass_utils import run_bass_kernel_spmd as _run_spmd
from concourse.bass_interp import get_hw_module as _get_hw_module

_MODULE_CACHE = {}


def _get_module(num_cores):
    key = num_cores
    if key not in _MODULE_CACHE:
        nc = _bacc.Bacc("TRN2", target_bir_lowering=False, debug=False,
                        enable_asserts=False, num_devices=num_cores)
        io = declare_io(nc, debug=False)
        with tile.TileContext(nc) as tc:
            build(tc, io)
        nc.compile()
        nc.m = _get_hw_module(nc.m)
        _MODULE_CACHE[key] = nc
    return _MODULE_CACHE[key]


def kernel(x, w_offset, w):
    """DeformConv: x [8,128,64,64] f32, w_offset [18,128,3,3] f32,
    w [128,128,3,3] f32 -> out [8,128,64,64] f32. One sample per NeuronCore."""
    x = np.ascontiguousarray(np.asarray(x), dtype=np.float32)
    w_offset = np.asarray(w_offset)
    w = np.asarray(w)
    B = x.shape[0]
    nc = _get_module(B)
    shared = {**host_weights(w_offset, w), **host_constants()}
    in_maps = [{"xin": x[b].reshape(128, HW), **shared} for b in range(B)]
    res = _run_spmd(nc, in_maps, core_ids=list(range(B)))
    out = np.stack([res.results[b]["out"].reshape(128, 64, 64) for b in range(B)])
    return out.astype(np.float32)



# revision 48
# speedup vs baseline: 1.6840x; 1.6840x over previous
"""Bass/Tile kernel for DeformConv (one sample per NeuronCore), v2.

Pipeline (per core, x = [128 c, 4096 l] with l = y*64 + x):
  1. offset conv (PE, fp16) -> offsb [18, 4096] -> transpose -> offT2 [128, 18, 32]
  2. batched coords (DVE, f32 [128, 9*32] ops): bilinear corner weights
     w4 [128 lp, 4 corner, 9 k, 32 i] and clamped pixel index pix16/pixbot16
  3. x -> xtd DRAM [4352, 128] f16 (row = pixel + 128, zero guard bands)
  4. per (hf, k): dma_gather of corner-row PAIRS: index j = g*128 + 2*m + s
     (g = 64-l group, s = 0 top / 1 bottom pair, m = l%64) -> partition 2m+s,
     elem 256 f16 = rows [idx, idx+1] = (left corner, right corner) channels
  5. per 64-l group: two tensor_scalar mults (4x DVE mode) scale the pairs by
     bilinear weights; two tiny PE matmuls against a static 0/1 selector
     SEL[p, m'] = (p//2 == m') contract the 4 corners AND transpose to
     psum[c, 64] = sampled^T
  6. Act evacuates psum -> sampN [c, 512] f16; main matmul accumulates
     out^T psum over the 9 taps.

Index identities (verified):
  l = hf*2048 + g_sub*64 + m ; l%128 = (g_sub%2)*64 + m ; l//128 = hf*16 + g_sub//2
  gather j = g_sub*128 + 2*m + spair -> partition p = 2m+spair, slot g_sub;
  idx slot (q = j%16 = (2m+spair)%16, s2 = j//16 = g_sub*8 + m//8), so with
  mh = m//8, ml = m%8: q = 2*ml + spair, s2 = g_sub*8 + mh -> the idx DMA from
  pixb2[part_old, spair, k, i] is affine with per-partition stride 288*q.
"""
from contextlib import ExitStack

import numpy as np

import concourse.bass as bass
import concourse.mybir as mybir
import concourse.tile as tile
from concourse import masks

F32 = mybir.dt.float32
F16 = mybir.dt.float16
I16 = mybir.dt.int16
I32 = mybir.dt.int32
ALU = mybir.AluOpType
ACTF = mybir.ActivationFunctionType

Cin = Cout = 128
HW = 4096
NTAP = 9
GUARD = 128
NROWS = HW + 2 * GUARD  # 4352
NBLK = 32   # 128-l blocks
NG = 64     # l's per gather group
NGRP = HW // NG  # 64
NHALF = 2
GPH = NGRP // NHALF  # 32 groups per half
KF = NTAP * NBLK     # 288, batched coord free size


def host_constants():
    l = np.arange(HW)
    p = l % 128
    blk = l // 128
    i_img = l // 64
    j_img = l % 64
    ky = np.arange(9) // 3
    kx = np.arange(9) % 3
    basepy = np.zeros((128, 9, 32), np.float32)
    basepx = np.zeros((128, 9, 32), np.float32)
    for k in range(9):
        basepy[p, k, blk] = i_img - 1 + ky[k]
        basepx[p, k, blk] = j_img - 1 + kx[k]
    sel = np.zeros((128, 64), np.float16)
    pp = np.arange(128)
    sel[pp, pp // 2] = 1.0
    return {"basepy": basepy, "basepx": basepx, "sel": sel}


def host_weights(w_offset, w):
    wofft = np.ascontiguousarray(
        w_offset.transpose(2, 3, 1, 0).reshape(9, 128, 18)).astype(np.float16)
    wmainT = np.ascontiguousarray(
        w.transpose(2, 3, 1, 0).reshape(9, 128, 128)).astype(np.float16)
    return {"wofft": wofft, "wmainT": wmainT}


def declare_io(nc, debug=False):
    io = {}
    io["xin"] = nc.dram_tensor("xin", (128, HW), F32, kind="ExternalInput").ap()
    io["wofft"] = nc.dram_tensor("wofft", (9, 128, 18), F16, kind="ExternalInput").ap()
    io["wmainT"] = nc.dram_tensor("wmainT", (9, 128, 128), F16, kind="ExternalInput").ap()
    io["basepy"] = nc.dram_tensor("basepy", (128, 9, 32), F32, kind="ExternalInput").ap()
    io["basepx"] = nc.dram_tensor("basepx", (128, 9, 32), F32, kind="ExternalInput").ap()
    io["sel"] = nc.dram_tensor("sel", (128, 64), F16, kind="ExternalInput").ap()
    io["out"] = nc.dram_tensor("out", (128, HW), F32, kind="ExternalOutput").ap()
    io["xtd"] = nc.dram_tensor("xtd", (NROWS, 128), F16,
                               kind="ExternalOutput" if debug else "Internal").ap()
    for hf in range(2):
        io[f"pixb2h{hf}"] = nc.dram_tensor(f"pixb2h{hf}", (128, 2, 9, 16), I16,
                                           kind="Internal").ap()
        io[f"idxdh{hf}"] = nc.dram_tensor(f"idxdh{hf}", (16, 9 * 256), I16,
                                          kind="Internal").ap()
        io[f"w4dh{hf}"] = nc.dram_tensor(f"w4dh{hf}", (128, 4 * 9 * 16), F32,
                                         kind="Internal").ap()
    io["debug"] = debug
    if debug:
        io["d_offsb"] = nc.dram_tensor("d_offsb", (18, HW), F32, kind="ExternalOutput").ap()
        io["d_gq"] = nc.dram_tensor("d_gq", (128, 32 * 256), F16, kind="ExternalOutput").ap()
        io["d_sampN"] = nc.dram_tensor("d_sampN", (128, 512), F16, kind="ExternalOutput").ap()
    return io


def build(tc: tile.TileContext, io: dict):
    nc = tc.nc
    xin, wofft, wmainT = io["xin"], io["wofft"], io["wmainT"]
    basepy, basepx, out, sel = io["basepy"], io["basepx"], io["out"], io["sel"]
    xtd = io["xtd"]

    ctx = ExitStack()
    const = ctx.enter_context(tc.tile_pool(name="const", bufs=1))
    persist = ctx.enter_context(tc.tile_pool(name="persist", bufs=1))
    coord = ctx.enter_context(tc.tile_pool(name="coord", bufs=2))
    evac = ctx.enter_context(tc.tile_pool(name="evac", bufs=3))

    ident32 = const.tile([128, 128], F32)
    masks.make_identity(nc, ident32[:])
    ident16 = const.tile([128, 128], F16)
    masks.make_identity(nc, ident16[:])
    zeros16 = const.tile([128, 128], F16)
    nc.vector.memset(zeros16[:], 0.0)
    sel_sb = const.tile([128, 64], F16)
    nc.sync.dma_start(out=sel_sb[:], in_=sel)

    xtd_flat = xtd.rearrange("r c -> (r c)")
    ng = GUARD * 128
    nc.sync.dma_start(out=xtd_flat[0:ng].rearrange("(p n) -> p n", p=128),
                      in_=zeros16[:])
    nc.sync.dma_start(out=xtd_flat[(GUARD + HW) * 128:].rearrange("(p n) -> p n", p=128),
                      in_=zeros16[:])

    xctx = ExitStack()
    xbuf = xctx.enter_context(tc.tile_pool(name="xbuf", bufs=1))
    xpad = xbuf.tile([128, 66, 66], F16)
    # zero only the border ring (interior is overwritten by the load)
    nc.vector.memset(xpad[:, 0, :], 0.0)
    nc.vector.memset(xpad[:, 65, :], 0.0)
    nc.vector.memset(xpad[:, 1:65, 0], 0.0)
    nc.vector.memset(xpad[:, 1:65, 65], 0.0)
    nc.gpsimd.dma_start(out=xpad[:, 1:65, 1:65],
                        in_=xin.rearrange("c (h w) -> c h w", h=64))
    # column-shifted contiguous copies: xsh[:, kx, r, j] = xpad[c, r, j+kx]
    xsh = xbuf.tile([128, 3, 66 * 64], F16)
    for kx in range(3):
        nc.vector.tensor_copy(
            xsh[:, kx, :].rearrange("p (r j) -> p r j", r=66),
            xpad[:, :, kx:kx + 64])

    wofft_sb = persist.tile([128, 9, 18], F16)
    nc.sync.dma_start(out=wofft_sb[:], in_=wofft.rearrange("k c f -> c k f"))
    wmainT_sb = persist.tile([128, 9, 128], F16)
    nc.sync.dma_start(out=wmainT_sb[:], in_=wmainT.rearrange("k c o -> c k o"))
    basepy_sb = persist.tile([128, 9, 32], F32)
    nc.sync.dma_start(out=basepy_sb[:], in_=basepy)
    basepx_sb = persist.tile([128, 9, 32], F32)
    nc.sync.dma_start(out=basepx_sb[:], in_=basepx)

    offsb = persist.tile([18, HW], F32)
    offT2 = persist.tile([128, 18, 32], F32)

    # ------- coords + bounces per half: i-slice [hf*16, hf*16+16) = hf's 2048 l's -------
    idxwh = []
    w4rh = []

    def do_coords(hf):
        i0 = hf * 16

        def tc16(tag, dt=F32):
            return coord.tile([128, NTAP, 16], dt, tag=tag, name=f"{tag}{hf}")

        py = tc16("py")
        nc.vector.tensor_tensor(py[:], offT2[:, 0:18:2, i0:i0 + 16],
                                basepy_sb[:, :, i0:i0 + 16], ALU.add)
        px = tc16("px")
        nc.vector.tensor_tensor(px[:], offT2[:, 1:18:2, i0:i0 + 16],
                                basepx_sb[:, :, i0:i0 + 16], ALU.add)

        def floorfrac(src, tagp):
            ti = tc16(tagp + "i", I32)
            nc.vector.tensor_copy(ti[:], src[:])
            tf = tc16(tagp + "f")
            nc.vector.tensor_copy(tf[:], ti[:])
            gt = tc16(tagp + "g")
            nc.vector.tensor_tensor(gt[:], tf[:], src[:], ALU.is_gt)
            fl = tc16(tagp + "fl")
            nc.vector.tensor_tensor(fl[:], tf[:], gt[:], ALU.subtract)
            fr = tc16(tagp + "fr")
            nc.vector.tensor_tensor(fr[:], src[:], fl[:], ALU.subtract)
            return fl, fr

        y0, fy = floorfrac(py, "yy")
        x0, fx = floorfrac(px, "xx")

        # pixel index first (it gates the idx bounce -> gathers); clamped so
        # all 4 corner rows stay inside xtd
        pixf = tc16("pixf")
        nc.vector.tensor_scalar(pixf[:], y0[:], 64.0, None, ALU.mult)
        nc.vector.tensor_tensor(pixf[:], pixf[:], x0[:], ALU.add)
        pt = tc16("pt")
        nc.vector.tensor_scalar(pt[:], pixf[:], -66.0, 4096.0, ALU.max, ALU.min)
        ptT = tc16("ptT")
        nc.vector.tensor_scalar(ptT[:], pt[:], float(GUARD), None, ALU.add)
        pix16 = tc16("pix16", I16)
        nc.vector.tensor_copy(pix16[:], ptT[:])
        ptB = tc16("ptB")
        nc.vector.tensor_scalar(ptB[:], pt[:], float(GUARD + 64), None, ALU.add)
        pixbot16 = tc16("pixbot16", I16)
        nc.vector.tensor_copy(pixbot16[:], ptB[:])
        pixb2 = io[f"pixb2h{hf}"]
        nc.sync.dma_start(out=pixb2[:, 0], in_=pix16[:])
        nc.sync.dma_start(out=pixb2[:, 1], in_=pixbot16[:])

        # idx bounce: idxd[q, k, s2] with s2 = ghi*16 + w, w = glow*8 + h;
        # src addr = 144*q + k*16 + ghi + 2304*w (merges to 3 AP entries)
        pixb2_flat = pixb2.rearrange("p s k i -> (p s k i)")
        src = bass.AP(pixb2_flat.tensor, 0,
                      [[144, 16], [16, 9], [1, 16], [2304, 16]])
        idxd_flat = io[f"idxdh{hf}"].rearrange("q n -> (q n)")
        dstv = bass.AP(idxd_flat.tensor, 0,
                       [[2304, 16], [256, 9], [16, 16], [1, 16]])
        with nc.allow_non_contiguous_dma(reason="16-wrap idx shuffle, one-time"):
            nc.sync.dma_start(out=dstv, in_=src)
        # replicate x8 into SBUF in one read (stride-0 replica dim on DRAM src)
        idxw = persist.tile([128, 9, 256], I16, name=f"idxw{hf}")
        rep = bass.AP(idxd_flat.tensor, 0, [[0, 8], [2304, 16], [1, 2304]])
        nc.sync.dma_start(out=idxw[:], in_=rep)
        idxwh.append(idxw)

        def wmask(src, lo, hi, tag):
            m1 = tc16(tag + "a")
            nc.vector.tensor_scalar(m1[:], src[:], float(lo), None, ALU.is_ge)
            m2 = tc16(tag + "b")
            nc.vector.tensor_scalar(m2[:], src[:], float(hi), None, ALU.is_le)
            m = tc16(tag)
            nc.vector.tensor_tensor(m[:], m1[:], m2[:], ALU.mult)
            return m

        my0 = wmask(y0, 0, 63, "my0")
        my1 = wmask(y0, -1, 62, "my1")
        mx0 = wmask(x0, 0, 63, "mx0")
        mx1 = wmask(x0, -1, 62, "mx1")

        a0 = tc16("a0")
        nc.vector.tensor_scalar(a0[:], fy[:], -1.0, 1.0, ALU.mult, ALU.add)
        nc.vector.tensor_tensor(a0[:], a0[:], my0[:], ALU.mult)
        a1 = tc16("a1")
        nc.vector.tensor_tensor(a1[:], fy[:], my1[:], ALU.mult)
        b0 = tc16("b0")
        nc.vector.tensor_scalar(b0[:], fx[:], -1.0, 1.0, ALU.mult, ALU.add)
        nc.vector.tensor_tensor(b0[:], b0[:], mx0[:], ALU.mult)
        b1 = tc16("b1")
        nc.vector.tensor_tensor(b1[:], fx[:], mx1[:], ALU.mult)

        # corner order: 0=TL(a0b0) 1=TR(a0b1) 2=BL(a1b0) 3=BR(a1b1)
        w4 = coord.tile([128, 4, NTAP, 16], F32, tag="w4", name=f"w4{hf}")
        nc.vector.tensor_tensor(w4[:, 0, :, :], a0[:], b0[:], ALU.mult)
        nc.vector.tensor_tensor(w4[:, 1, :, :], a0[:], b1[:], ALU.mult)
        nc.vector.tensor_tensor(w4[:, 2, :, :], a1[:], b0[:], ALU.mult)
        nc.vector.tensor_tensor(w4[:, 3, :, :], a1[:], b1[:], ALU.mult)

        w4d = io[f"w4dh{hf}"]
        nc.sync.dma_start(out=w4d, in_=w4[:].rearrange("p a b c -> p (a b c)"))
        w4d_flat = w4d.rearrange("p n -> (p n)")
        # w4r[p, lr, glow, k, ghi] <- w4d[part_old=(glow*64+p//2),
        # corner=(p%2)*2+lr, k, ghi]; per-partition offset = 288*p (affine)
        w4r = persist.tile([128, 2, 2, NTAP, 16], F32, name=f"w4r{hf}")
        for lr in range(2):
            for glow in range(2):
                srcw = bass.AP(w4d_flat.tensor, lr * 144 + glow * 36864,
                               [[288, 128], [16, 9], [1, 16]])
                nc.sync.dma_start(out=w4r[:, lr, glow], in_=srcw)
        w4rh.append(w4r)

    if io["debug"]:
        nc.sync.dma_start(out=io["d_offsb"], in_=offsb[:])


    # ---------------- prologue (own PSUM scope) ----------------
    # Emission order matters: PE queues are in-order, and the first gather
    # needs BOTH xtd complete and the hf0 idx pipeline, so xtd transposes go
    # first, then conv/offT2/coords interleaved per half.
    with tc.tile_pool(name="prepsum", bufs=2, space="PSUM") as pps:
        # xtd build (fp16 transpose): block i covers pixels [i*128, (i+1)*128)
        for ib in range(4):
            xts = evac.tile([128, 8, 128], F16, tag="xts")
            for j in range(8):
                i = ib * 8 + j
                psx = pps.tile([128, 128], F16, tag="psx")
                r0 = (2 * i + 1) * 64
                nc.tensor.transpose(psx[:], xsh[:, 1, r0:r0 + 128], ident16[:])
                nc.scalar.activation(xts[:, j, :], psx[:], ACTF.Copy)
            dst = bass.AP(xtd_flat.tensor, (GUARD + 1024 * ib) * 128,
                          [[128, 128], [16384, 8], [1, 128]])
            nc.sync.dma_start(out=dst, in_=xts[:])

        for hf in range(NHALF):
            # offset conv for this half (fp16 in, fp32 psum)
            for nb in range(4 * hf, 4 * hf + 4):
                ps = pps.tile([18, 512], F32, tag="psoff")
                for k in range(NTAP):
                    ky, kx = k // 3, k % 3
                    r0 = (nb * 8 + ky) * 64
                    rhs = xsh[:, kx, r0:r0 + 512]
                    nc.tensor.matmul(ps[:], wofft_sb[:, k, :], rhs,
                                     start=(k == 0), stop=(k == NTAP - 1))
                nc.scalar.activation(offsb[:, nb * 512:(nb + 1) * 512], ps[:],
                                     ACTF.Copy)
            # transpose offsets -> offT2 [128, 18 f, 32 i]
            for i in range(16 * hf, 16 * hf + 16):
                pst = pps.tile([128, 18], F32, tag="pst")
                nc.tensor.transpose(pst[:], offsb[:, i * 128:(i + 1) * 128],
                                    ident32[0:18, 0:18])
                nc.scalar.activation(offT2[:, :, i], pst[:], ACTF.Copy)
            do_coords(hf)

    xctx.close()  # free xpad/xsh before the main loop needs SBUF

    # ---------------- main loop ----------------
    gather_src = bass.AP(xtd_flat.tensor, 0, [[128, NROWS - 1], [1, 256]])

    with tc.tile_pool(name="psout", bufs=1, space="PSUM") as psout, \
         tc.tile_pool(name="psel", bufs=3, space="PSUM") as psel, \
         tc.tile_pool(name="gpool", bufs=3) as gpool, \
         tc.tile_pool(name="qpool", bufs=6) as qpool, \
         tc.tile_pool(name="spool", bufs=2) as spool:
        for hf in range(NHALF):
            pso = [psout.tile([128, 512], F32, tag=f"pso{c}", name=f"pso{c}_{hf}")
                   for c in range(4)]
            for k in range(NTAP):
                gq = gpool.tile([128, GPH, 256], F16, tag="gq")
                nc.gpsimd.dma_gather(
                    out_ap=gq[:],
                    in_ap=gather_src,
                    idxs_ap=idxwh[hf][:, k, :],
                    num_idxs=GPH * 128,
                    num_idxs_reg=GPH * 128,
                    elem_size=256,
                    elem_step=128,
                    single_packet=False,
                )
                for c in range(4):
                    ps = psel.tile([128, 512], F32, tag="psel")
                    for j in range(8):
                        g_sub = c * 8 + j
                        g = hf * GPH + g_sub
                        gqs = qpool.tile([128, 256], F16, tag="gqs")
                        sL = w4rh[hf][:, 0, g_sub % 2, k,
                                      g_sub // 2:g_sub // 2 + 1]
                        sR = w4rh[hf][:, 1, g_sub % 2, k,
                                      g_sub // 2:g_sub // 2 + 1]
                        nc.vector.tensor_scalar(gqs[:, 0:128], gq[:, g_sub, 0:128],
                                                sL, None, ALU.mult)
                        nc.vector.tensor_scalar(gqs[:, 128:256], gq[:, g_sub, 128:256],
                                                sR, None, ALU.mult)
                        nc.tensor.matmul(ps[:, j * 64:(j + 1) * 64], gqs[:, 0:128],
                                         sel_sb[:], start=True, stop=False)
                        nc.tensor.matmul(ps[:, j * 64:(j + 1) * 64], gqs[:, 128:256],
                                         sel_sb[:], start=False, stop=True)
                    sampN = spool.tile([128, 512], F16, tag="sampN")
                    nc.scalar.activation(sampN[:], ps[:], ACTF.Copy)
                    if io["debug"] and hf == 0 and k == 0 and c == 0:
                        nc.sync.dma_start(out=io["d_gq"],
                                          in_=gq[:].rearrange("p a b -> p (a b)"))
                        nc.sync.dma_start(out=io["d_sampN"], in_=sampN[:])
                    nc.tensor.matmul(pso[c][:], wmainT_sb[:, k, :], sampN[:],
                                     start=(k == 0), stop=(k == NTAP - 1))
            for c in range(4):
                osb = evac.tile([128, 512], F32, tag="osb")
                nc.scalar.activation(osb[:], pso[c][:], ACTF.Copy)
                l0 = hf * 2048 + c * 512
                nc.sync.dma_start(out=out[:, l0:l0 + 512], in_=osb[:])
    ctx.close()


# ======================= runner =======================
import concourse.bacc as _bacc
from concourse.bass_utils import run_bass_kernel_spmd as _run_spmd
from concourse.bass_interp import get_hw_module as _get_hw_module

_MODULE_CACHE = {}


def make_nc(num_cores):
    return _bacc.Bacc("TRN2", target_bir_lowering=False, debug=False,
                      enable_asserts=False, num_devices=num_cores,
                      dynamic_dma_scratch_size=65536, num_swdge_queues=2)


def _get_module(num_cores):
    key = num_cores
    if key not in _MODULE_CACHE:
        nc = make_nc(num_cores)
        io = declare_io(nc, debug=False)
        with tile.TileContext(nc) as tc:
            build(tc, io)
        nc.compile()
        nc.m = _get_hw_module(nc.m)
        _MODULE_CACHE[key] = nc
    return _MODULE_CACHE[key]


def kernel(x, w_offset, w):
    """DeformConv: x [8,128,64,64] f32, w_offset [18,128,3,3] f32,
    w [128,128,3,3] f32 -> out [8,128,64,64] f32. One sample per NeuronCore."""
    x = np.ascontiguousarray(np.asarray(x), dtype=np.float32)
    w_offset = np.asarray(w_offset)
    w = np.asarray(w)
    B = x.shape[0]
    nc = _get_module(B)
    shared = {**host_weights(w_offset, w), **host_constants()}
    in_maps = [{"xin": x[b].reshape(128, HW), **shared} for b in range(B)]
    res = _run_spmd(nc, in_maps, core_ids=list(range(B)))
    out = np.stack([res.results[b]["out"].reshape(128, 64, 64) for b in range(B)])
    return out.astype(np.float32)
